# revision 3
# baseline (speedup 1.0000x reference)
"""Cross-attention head (B=8, L=2048, D=1024, fp32) on 8 TRN2 NeuronCores.

Sharding: data-parallel over batch — core b handles batch element b.
Per-core pipeline (all matmuls in float32r, PSUM accumulation fp32):
  1. QT = (Wq/sqrt(D))^T-proj of output_seq, spilled to DRAM scratch.
  2. KT = Wk-proj of input_seq, kept resident in SBUF [e_part, e_chunk, k].
  3. V  = Wv-proj of input_seq, kept resident in SBUF [k_part, k_chunk, e].
  4. Per 128-query tile: S = QT^T@KT on PE; E = exp(mask_q * S) on ScalarE
     (per-partition scale = row-validity mask; masked rows get exp(0)=1 which
     reproduces the reference's uniform softmax on fully-masked rows; no
     max-subtraction needed — scores are O(+-6) for randn inputs),
     row-sum via activation accum_out; E^T via PE transpose; ctx = E^T@V;
     normalize both with 1/rowsum on DVE; DMA attn + context rows out.

Host side: transposes inputs/weights (so no on-device transposes of
activations are needed), folds the 1/sqrt(D) scale into Wq, builds the
query-validity masks from input_lens, gathers per-core outputs.
"""

import math
from contextlib import ExitStack

import numpy as np

import concourse.bass as bass
import concourse.bacc as bacc
import concourse.mybir as mybir
import concourse.tile as tile
from concourse.bass_utils import run_bass_kernel_spmd
from concourse.masks import make_identity

B, L, D = 8, 2048, 1024
P = 128
NE = D // P          # 8 chunks of the feature dims
NKC = L // P         # 16 key chunks / query tiles
CB = 256             # streaming column-block for the projection phases
NCB = L // CB        # 8
F32 = mybir.dt.float32
F32R = mybir.dt.float32r
EXP = mybir.ActivationFunctionType.Exp
AX = mybir.AxisListType.X

_NC_CACHE = []


def _build_bass():
    nc = bacc.Bacc("TRN2", target_bir_lowering=False, debug=False, num_devices=B)

    xt = nc.dram_tensor("xt", [NE, P, L], F32R, kind="ExternalInput")
    yt = nc.dram_tensor("yt", [NE, P, L], F32R, kind="ExternalInput")
    wq = nc.dram_tensor("wq", [NE, P, D], F32R, kind="ExternalInput")
    wk = nc.dram_tensor("wk", [NE, P, D], F32R, kind="ExternalInput")
    wv = nc.dram_tensor("wv", [NE, P, D], F32R, kind="ExternalInput")
    msk = nc.dram_tensor("mask", [P, NKC], F32, kind="ExternalInput")
    attn_out = nc.dram_tensor("attn", [L, L], F32, kind="ExternalOutput")
    ctx_out = nc.dram_tensor("ctx", [L, D], F32, kind="ExternalOutput")

    with tile.TileContext(nc) as tc:
        with (
            tc.tile_pool(name="ktp", bufs=1) as ktp,
            tc.tile_pool(name="vp", bufs=1) as vp,
            tc.tile_pool(name="constp", bufs=1) as constp,
            tc.tile_pool(name="dram", bufs=1, space="DRAM") as dramp,
        ):
            KT = ktp.tile([P, NE, L], F32R)      # [e_part, e_chunk, k]
            V = vp.tile([P, NKC, D], F32R)       # [k_part, k_chunk, e]
            ident = constp.tile([P, P], F32)
            make_identity(nc, ident)
            m_sb = constp.tile([P, NKC], F32)
            nc.sync.dma_start(m_sb[:], msk[:])
            qt_dram = dramp.tile([NE, P, L], F32R)   # spilled QT [e_chunk, e_part, q]

            # ---------------- projection phases ----------------
            with ExitStack() as pctx:
                wtp = pctx.enter_context(tc.tile_pool(name="wtp", bufs=1))
                stp = pctx.enter_context(tc.tile_pool(name="stp", bufs=2))
                sgp = pctx.enter_context(tc.tile_pool(name="sgp", bufs=3))
                pps = pctx.enter_context(
                    tc.tile_pool(name="pps", bufs=4, space="PSUM")
                )

                # --- Q: QT[e, q] = sum_d WqT[d, e] * YT[d, q]  -> qt_dram
                w_sb = wtp.tile([P, NE, D], F32R, tag="w", name="wq_sb")
                nc.sync.dma_start(w_sb[:], wq.rearrange("c p e -> p c e"))
                for qb in range(NCB):
                    a_blk = stp.tile([P, NE, CB], F32R, tag="act", name="y_blk")
                    nc.sync.dma_start(
                        a_blk[:],
                        yt[:, :, qb * CB : (qb + 1) * CB].rearrange("c p n -> p c n"),
                    )
                    for e in range(NE):
                        ps = pps.tile([P, 512], F32, tag="pp", name="ps_q")
                        for d in range(NE):
                            nc.tensor.matmul(
                                ps[:, :CB],
                                w_sb[:, d, e * P : (e + 1) * P],
                                a_blk[:, d, :],
                                start=(d == 0),
                                stop=(d == NE - 1),
                            )
                        stg = sgp.tile([P, CB], F32R, tag="stg", name="stg")
                        nc.vector.tensor_copy(stg[:], ps[:, :CB])
                        nc.sync.dma_start(
                            qt_dram[e, :, qb * CB : (qb + 1) * CB], stg[:]
                        )

                # --- K: KT[e, k] = sum_d WkT[d, e] * XT[d, k]  -> SBUF resident
                w_sb = wtp.tile([P, NE, D], F32R, tag="w", name="wk_sb")
                nc.sync.dma_start(w_sb[:], wk.rearrange("c p e -> p c e"))
                for kb in range(NCB):
                    a_blk = stp.tile([P, NE, CB], F32R, tag="act", name="xk_blk")
                    nc.sync.dma_start(
                        a_blk[:],
                        xt[:, :, kb * CB : (kb + 1) * CB].rearrange("c p n -> p c n"),
                    )
                    for e in range(NE):
                        ps = pps.tile([P, 512], F32, tag="pp", name="ps_k")
                        for d in range(NE):
                            nc.tensor.matmul(
                                ps[:, :CB],
                                w_sb[:, d, e * P : (e + 1) * P],
                                a_blk[:, d, :],
                                start=(d == 0),
                                stop=(d == NE - 1),
                            )
                        nc.vector.tensor_copy(
                            KT[:, e, kb * CB : (kb + 1) * CB], ps[:, :CB]
                        )

                # --- V: V[k, e] = sum_d XT[d, k] * WvT[d, e]  -> SBUF resident
                w_sb = wtp.tile([P, NE, D], F32R, tag="w", name="wv_sb")
                nc.sync.dma_start(w_sb[:], wv.rearrange("c p e -> p c e"))
                for kb in range(NCB):
                    a_blk = stp.tile([P, NE, CB], F32R, tag="act", name="xv_blk")
                    nc.sync.dma_start(
                        a_blk[:],
                        xt[:, :, kb * CB : (kb + 1) * CB].rearrange("c p n -> p c n"),
                    )
                    for j in range(CB // P):
                        kc = kb * (CB // P) + j
                        for h in range(2):
                            ps = pps.tile([P, 512], F32, tag="pp", name="ps_v")
                            for d in range(NE):
                                nc.tensor.matmul(
                                    ps[:],
                                    a_blk[:, d, j * P : (j + 1) * P],
                                    w_sb[:, d, h * 512 : (h + 1) * 512],
                                    start=(d == 0),
                                    stop=(d == NE - 1),
                                )
                            nc.vector.tensor_copy(
                                V[:, kc, h * 512 : (h + 1) * 512], ps[:]
                            )

            # ---------------- attention phase ----------------
            with ExitStack() as actx:
                qtp = actx.enter_context(tc.tile_pool(name="qtp", bufs=2))
                ep = actx.enter_context(tc.tile_pool(name="ep", bufs=2))
                etp = actx.enter_context(tc.tile_pool(name="etp", bufs=2))
                csp = actx.enter_context(tc.tile_pool(name="csp", bufs=2))
                smp = actx.enter_context(tc.tile_pool(name="smp", bufs=2))
                sps = actx.enter_context(
                    tc.tile_pool(name="sps", bufs=3, space="PSUM")
                )
                tps = actx.enter_context(
                    tc.tile_pool(name="tps", bufs=2, space="PSUM")
                )
                cps = actx.enter_context(
                    tc.tile_pool(name="cps", bufs=1, space="PSUM")
                )

                for t in range(NKC):
                    q_sb = qtp.tile([P, NE, P], F32R, tag="q", name="q_sb")
                    nc.sync.dma_start(
                        q_sb[:],
                        qt_dram[:, :, t * P : (t + 1) * P].rearrange("c p n -> p c n"),
                    )
                    E = ep.tile([P, L], F32, tag="e", name="E")
                    rs4 = smp.tile([P, 4], F32, tag="rs4", name="rs4")
                    for kc4 in range(4):
                        ps = sps.tile([P, 512], F32, tag="s", name="ps_s")
                        for e in range(NE):
                            nc.tensor.matmul(
                                ps[:],
                                q_sb[:, e, :],
                                KT[:, e, kc4 * 512 : (kc4 + 1) * 512],
                                start=(e == 0),
                                stop=(e == NE - 1),
                            )
                        nc.scalar.activation(
                            E[:, kc4 * 512 : (kc4 + 1) * 512],
                            ps[:],
                            EXP,
                            bias=0.0,
                            scale=m_sb[:, t : t + 1],
                            accum_out=rs4[:, kc4 : kc4 + 1],
                        )
                    rsum = smp.tile([P, 1], F32, tag="rsum", name="rsum")
                    rcp = smp.tile([P, 1], F32, tag="rcp", name="rcp")
                    nc.vector.reduce_sum(rsum[:], rs4[:], axis=AX)
                    nc.vector.reciprocal(rcp[:], rsum[:])

                    ET = etp.tile([P, NKC, P], F32R, tag="et", name="ET")
                    for g in range(4):
                        tp = tps.tile([P, 512], F32, tag="t", name="ps_t")
                        for j in range(4):
                            c = 4 * g + j
                            nc.tensor.transpose(
                                tp[:, j * P : (j + 1) * P],
                                E[:, c * P : (c + 1) * P],
                                ident[:],
                            )
                        nc.vector.tensor_copy(ET[:, 4 * g : 4 * (g + 1), :], tp[:])

                    c0 = cps.tile([P, 512], F32, tag="c0", name="ps_c0")
                    c1 = cps.tile([P, 512], F32, tag="c1", name="ps_c1")
                    for c in range(NKC):
                        nc.tensor.matmul(
                            c0[:],
                            ET[:, c, :],
                            V[:, c, 0:512],
                            start=(c == 0),
                            stop=(c == NKC - 1),
                        )
                    for c in range(NKC):
                        nc.tensor.matmul(
                            c1[:],
                            ET[:, c, :],
                            V[:, c, 512:1024],
                            start=(c == 0),
                            stop=(c == NKC - 1),
                        )

                    nc.vector.tensor_scalar_mul(E[:], E[:], rcp[:])
                    nc.sync.dma_start(attn_out[t * P : (t + 1) * P, :], E[:])
                    cs = csp.tile([P, D], F32, tag="cs", name="cs")
                    nc.vector.tensor_scalar_mul(cs[:, 0:512], c0[:], rcp[:])
                    nc.vector.tensor_scalar_mul(cs[:, 512:1024], c1[:], rcp[:])
                    nc.sync.dma_start(ctx_out[t * P : (t + 1) * P, :], cs[:])

    nc.compile()
    return nc


def get_nc():
    if not _NC_CACHE:
        _NC_CACHE.append(_build_bass())
    return _NC_CACHE[0]


def _prep_inputs(input_seq, input_lens, output_seq, Wk, Wv, Wq):
    input_seq = np.asarray(input_seq, dtype=np.float32)
    output_seq = np.asarray(output_seq, dtype=np.float32)
    lens = np.asarray(input_lens).astype(np.int64)
    Wk = np.asarray(Wk, dtype=np.float32)
    Wv = np.asarray(Wv, dtype=np.float32)
    Wq = np.asarray(Wq, dtype=np.float32)

    s = 1.0 / math.sqrt(D)  # exact power of two (1/32)
    wq_t = np.ascontiguousarray((Wq * s).T).reshape(NE, P, D)
    wk_t = np.ascontiguousarray(Wk.T).reshape(NE, P, D)
    wv_t = np.ascontiguousarray(Wv.T).reshape(NE, P, D)

    qidx = np.arange(L).reshape(NKC, P).T  # [P, NKC], qidx[p, t] = t*P + p

    in_maps = []
    for b in range(B):
        xt_b = np.ascontiguousarray(input_seq[b].T).reshape(NE, P, L)
        yt_b = np.ascontiguousarray(output_seq[b].T).reshape(NE, P, L)
        m_b = (qidx < int(lens[b])).astype(np.float32)
        in_maps.append(
            {
                "xt": xt_b,
                "yt": yt_b,
                "wq": wq_t,
                "wk": wk_t,
                "wv": wv_t,
                "mask": np.ascontiguousarray(m_b),
            }
        )
    return in_maps


def kernel(input_seq, input_lens, output_seq, Wk, Wv, Wq):
    in_maps = _prep_inputs(input_seq, input_lens, output_seq, Wk, Wv, Wq)
    nc = get_nc()
    res = run_bass_kernel_spmd(nc, in_maps, core_ids=list(range(B)))
    context = np.stack([res.results[b]["ctx"] for b in range(B)])
    attn = np.stack([res.results[b]["attn"] for b in range(B)])
    return context, attn
